# revision 36
# baseline (speedup 1.0000x reference)
"""Trainium2 Bass kernel: causal multi-head self-attention block (pre-LN).

Full module computed on 8 NeuronCores:
    xn = LayerNorm(x); q = xn@Wq.T+bq; k,v = xn@Wkv.T+bkv
    out = softmax(mask(q k^T / sqrt(dh))) v @ Wo.T + bo + x

Sharding: core = batch_index * 2 + head_half.  Each core handles one batch
element and 8 of the 16 heads (column-parallel QKV, row-parallel Wo), emits a
partial [S, D] output including half the residual; host sums core pairs and
adds bo.  Weights are pre-transposed and cast to bf16 on the host so they DMA
straight into the matmul-ready layout.

Shapes are hardcoded for B=4, S=2048, D=1024, H=16, DH=64.
"""

import os
import sys

import numpy as np

sys.path.insert(0, "/opt/trn_rl_repo")

B, S, D, H = 4, 2048, 1024, 16
DH = D // H            # 64
HL = H // 2            # heads per core: 8
OH = HL * DH           # per-core head features: 512
EPS = 1e-5
NEG = -30000.0         # additive mask; exp(x + NEG) underflows to 0
P = 128                # SBUF partitions
NST = S // P           # 16 s-tiles
NFT = D // P           # 8 feature tiles
NOT = OH // P          # 4 o-tiles (per-core head features)
QS = 512               # query super-tile (matmul moving free dim)
NQS = S // QS          # 4
KT_PER_QS = QS // P    # 4 k-tiles per q-super

_CACHE = {}


def _build_nc():
    import concourse.bass as bass
    import concourse.bacc as bacc
    import concourse.tile as tile
    from concourse import mybir

    f32 = mybir.dt.float32
    bf16 = mybir.dt.bfloat16
    Alu = mybir.AluOpType
    Act = mybir.ActivationFunctionType

    nc = bacc.Bacc("TRN2", target_bir_lowering=False, debug=False, num_devices=8)

    # ---- DRAM I/O (per-core shard shapes; w* pre-transposed + bf16 on host) ----
    x_d = nc.dram_tensor("x", [S, D], f32, kind="ExternalInput").ap()
    wq_d = nc.dram_tensor("wqt", [D, OH], bf16, kind="ExternalInput").ap()
    wk_d = nc.dram_tensor("wkt", [D, OH], bf16, kind="ExternalInput").ap()
    wv_d = nc.dram_tensor("wvt", [D, OH], bf16, kind="ExternalInput").ap()
    wo_d = nc.dram_tensor("wot", [OH, D], bf16, kind="ExternalInput").ap()
    bq_d = nc.dram_tensor("bq", [OH], f32, kind="ExternalInput").ap()
    bk_d = nc.dram_tensor("bk", [OH], f32, kind="ExternalInput").ap()
    bv_d = nc.dram_tensor("bv", [OH], f32, kind="ExternalInput").ap()
    pad_d = nc.dram_tensor("pad01", [P, NST], f32, kind="ExternalInput").ap()
    out_d = nc.dram_tensor("out", [S, D], f32, kind="ExternalOutput").ap()
    debug = bool(os.environ.get("KERNEL_DEBUG"))
    if debug:
        dbg = {n: nc.dram_tensor(f"dbg_{n}", shp, bf16, kind="ExternalOutput").ap()
               for n, shp in (("xnT0", [P, S]), ("qT0", [P, S]), ("kT0", [P, S]),
                              ("vaug0", [P, HL * (DH + 1)]), ("oT0", [P, S]),
                              ("wqT0", [P, OH]))}

    def bcast(ap_1d, n):
        # [n] dram vector -> [P, n] partition-broadcast DMA source
        return bass.AP(tensor=ap_1d.tensor, offset=ap_1d.offset,
                       ap=[[0, P], [1, n]])

    with tile.TileContext(nc) as tc:
        with (
            tc.tile_pool(name="res", bufs=1) as res,       # resident tensors
            tc.tile_pool(name="small", bufs=4) as small,
        ):
            # ---------- constants ----------
            vb_sb = res.tile([P, OH], f32, tag="vb_sb")
            nc.sync.dma_start(out=vb_sb, in_=bcast(bv_d, OH))
            pad_sb = res.tile([P, NST], f32, tag="pad_sb")
            nc.sync.dma_start(out=pad_sb, in_=pad_d)
            zero_sb = res.tile([P, 1], f32, tag="zero_sb")
            nc.vector.memset(zero_sb, 0.0)
            ident_b = res.tile([P, P], bf16, tag="ident_b")
            nc.gpsimd.memset(ident_b, 0.0)
            nc.gpsimd.affine_select(
                out=ident_b, in_=ident_b, compare_op=Alu.not_equal, fill=1.0,
                base=0, pattern=[[-1, P]], channel_multiplier=1)
            bq_sb = res.tile([P, NOT], f32, tag="bq_sb")
            nc.sync.dma_start(out=bq_sb, in_=bq_d.rearrange("(t p) -> p t", p=P))
            bk_sb = res.tile([P, NOT], f32, tag="bk_sb")
            nc.sync.dma_start(out=bk_sb, in_=bk_d.rearrange("(t p) -> p t", p=P))
            eps_sb = res.tile([P, 1], f32, tag="eps_sb")
            nc.vector.memset(eps_sb, EPS)

            # ---------- resident big tensors ----------
            xnT = [res.tile([P, S], bf16, tag=f"xnT{j}", name=f"xnT{j}")
                   for j in range(NFT)]
            qT = [res.tile([P, S], bf16, tag=f"qT{t}", name=f"qT{t}")
                  for t in range(NOT)]
            kT = [res.tile([P, S], bf16, tag=f"kT{t}", name=f"kT{t}")
                  for t in range(NOT)]
            # V augmented with a ones column per head: [s, h*65 .. h*65+64]
            vaug = [res.tile([P, HL * (DH + 1)], bf16, tag=f"vaug{i}",
                             name=f"vaug{i}") for i in range(NST)]
            oT = [res.tile([P, S], bf16, tag=f"oT{t}", name=f"oT{t}")
                  for t in range(NOT)]
            wqT = [res.tile([P, OH], bf16, tag=f"wqT{j}", name=f"wqT{j}")
                   for j in range(NFT)]
            wkT = [res.tile([P, OH], bf16, tag=f"wkT{j}", name=f"wkT{j}")
                   for j in range(NFT)]
            wvT = [res.tile([P, OH], bf16, tag=f"wvT{j}", name=f"wvT{j}")
                   for j in range(NFT)]
            woT = [res.tile([P, D], bf16, tag=f"woT{t}", name=f"woT{t}")
                   for t in range(NOT)]

            # ---------- phases C/D/E interleaved ----------
            # s-chunk-major projections, then per-q-super attention + output
            # projection, so PE always has dense independent work in flight.
            with (
                tc.tile_pool(name="pj_psum", bufs=2, space="PSUM") as pp,
                tc.tile_pool(name="s_psum", bufs=2, space="PSUM") as sp,
                tc.tile_pool(name="o_psum", bufs=2, space="PSUM") as op,
                tc.tile_pool(name="pt", bufs=6) as ptp,
                tc.tile_pool(name="nrm", bufs=2) as nrm,
                tc.tile_pool(name="ld", bufs=3) as ld,
                tc.tile_pool(name="tmp", bufs=3) as tmp,
                tc.tile_pool(name="lde", bufs=3) as lde,
                tc.tile_pool(name="tmpe", bufs=3) as tmpe,
            ):
                for st in range(NST):
                    nc.gpsimd.memset(vaug[st], 1.0)
                def ln_compute(st, xns):
                    x_t = ld.tile([P, D], f32, tag="x_ln")
                    nc.sync.dma_start(out=x_t,
                                      in_=x_d[st * P:(st + 1) * P, :])
                    stats = small.tile([P, 2, 6], f32, tag="stats")
                    for sg in range(2):
                        nc.vector.bn_stats(
                            out=stats[:, sg, :],
                            in_=x_t[:, sg * 512:(sg + 1) * 512])
                    mv = small.tile([P, 2], f32, tag="mv")
                    nc.vector.bn_aggr(out=mv, in_=stats)
                    rstd = small.tile([P, 1], f32, tag="rstd")
                    nc.scalar.activation(out=rstd, in_=mv[:, 1:2],
                                         func=Act.Sqrt, bias=eps_sb,
                                         scale=1.0)
                    nc.vector.reciprocal(out=rstd, in_=rstd)
                    mb = small.tile([P, 1], f32, tag="mb")
                    nc.vector.tensor_scalar(
                        out=mb, in0=mv[:, 0:1], scalar1=rstd, scalar2=-1.0,
                        op0=Alu.mult, op1=Alu.mult)
                    xn = tmp.tile([P, D], bf16, tag="xn", bufs=9)
                    nc.scalar.activation(out=xn, in_=x_t, func=Act.Identity,
                                         bias=mb, scale=rstd)
                    xns[st] = xn

                def ln_transpose(st, xns):
                    xn = xns[st]
                    for j in range(NFT):
                        ps = pp.tile([P, P], bf16, tag="pj")
                        nc.tensor.transpose(
                            ps, xn[:, j * P:(j + 1) * P], ident_b)
                        nc.scalar.copy(
                            out=xnT[j][:, st * P:(st + 1) * P], in_=ps)

                xns = {}
                outproj_q = []
                for st in range(2 * KT_PER_QS):
                    ln_compute(st, xns)
                # weights DMA after the first LN x-loads so the prologue's
                # critical path isn't queued behind 5MB of weights
                for j in range(NFT):
                    nc.sync.dma_start(out=wqT[j], in_=wq_d[j * P:(j + 1) * P, :])
                    nc.sync.dma_start(out=wkT[j], in_=wk_d[j * P:(j + 1) * P, :])
                    nc.sync.dma_start(out=wvT[j], in_=wv_d[j * P:(j + 1) * P, :])
                for t in range(NOT):
                    nc.sync.dma_start(out=woT[t], in_=wo_d[t * P:(t + 1) * P, :])

                for c in range(NQS):
                    for st in range(c * KT_PER_QS, (c + 1) * KT_PER_QS):
                        ln_transpose(st, xns)
                    # --- projections for s-range [c*512, (c+1)*512) ---
                    for (wT, dst, bias) in ((wqT, qT, bq_sb), (wkT, kT, bk_sb)):
                        for t in range(NOT):
                            ps = pp.tile([P, QS], f32, tag="pj")
                            for j in range(NFT):
                                nc.tensor.matmul(
                                    ps,
                                    lhsT=wT[j][:, t * P:(t + 1) * P],
                                    rhs=xnT[j][:, c * QS:(c + 1) * QS],
                                    start=(j == 0), stop=(j == NFT - 1))
                            nc.vector.tensor_scalar_add(
                                out=dst[t][:, c * QS:(c + 1) * QS],
                                in0=ps, scalar1=bias[:, t:t + 1])
                    for st in range(c * KT_PER_QS, (c + 1) * KT_PER_QS):
                        ps = pp.tile([P, OH], f32, tag="pj")
                        for j in range(NFT):
                            nc.tensor.matmul(
                                ps,
                                lhsT=xnT[j][:, st * P:(st + 1) * P],
                                rhs=wvT[j],
                                start=(j == 0), stop=(j == NFT - 1))
                        for h in range(HL):
                            nc.vector.tensor_add(
                                out=vaug[st][:, h * (DH + 1):h * (DH + 1) + DH],
                                in0=ps[:, h * DH:(h + 1) * DH],
                                in1=vb_sb[:, h * DH:(h + 1) * DH])
                        nc.vector.tensor_scalar_mul(
                            out=vaug[st], in0=vaug[st],
                            scalar1=pad_sb[:, st:st + 1])

                    # LN for chunk c+2 races on DVE/GpSimd during attention
                    if c + 2 < NQS:
                        for st in range((c + 2) * KT_PER_QS,
                                        (c + 3) * KT_PER_QS):
                            ln_compute(st, xns)

                    # --- attention for qs=c, two heads' chains interleaved ---
                    qs = c
                    nkt = (qs + 1) * KT_PER_QS

                    def qk(h, kt, s_ps):
                        hq = qT[h // 2][(h % 2) * DH:(h % 2) * DH + DH, :]
                        hk = kT[h // 2][(h % 2) * DH:(h % 2) * DH + DH, :]
                        nc.tensor.matmul(
                            s_ps,
                            lhsT=hk[:, kt * P:(kt + 1) * P],
                            rhs=hq[:, qs * QS:(qs + 1) * QS],
                            start=True, stop=True, skip_group_check=True)

                    def expmask(kt0, s_ps):
                        pt = ptp.tile([P, 2, QS], bf16, tag="pt", bufs=4)
                        nc.scalar.activation(
                            out=pt, in_=s_ps, func=Act.Exp,
                            bias=zero_sb, scale=0.125)
                        for i in range(2):
                            kt = kt0 + i
                            if kt >= qs * KT_PER_QS:  # diagonal region
                                nc.gpsimd.affine_select(
                                    out=pt[:, i, :], in_=pt[:, i, :],
                                    compare_op=Alu.is_ge, fill=0.0,
                                    base=qs * QS - kt * P,
                                    pattern=[[1, QS]], channel_multiplier=-1)
                        return pt

                    def av(h, kt0, pt, o_ps):
                        for i in range(2):
                            kt = kt0 + i
                            nc.tensor.matmul(
                                o_ps,
                                lhsT=vaug[kt][:, h * (DH + 1):(h + 1) * (DH + 1)],
                                rhs=pt[:, i, :],
                                start=(kt == 0), stop=(kt == nkt - 1),
                                skip_group_check=True)

                    def normalize(h, o_ps):
                        den_sb = nrm.tile([1, QS], f32, tag="den_sb")
                        nc.vector.tensor_copy(den_sb, o_ps[DH:DH + 1, :])
                        dbc = nrm.tile([DH, QS], f32, tag="dbc")
                        nc.vector.reciprocal_approx_fast(
                            out=dbc[0:1, :], in_=den_sb)
                        nc.gpsimd.partition_broadcast(dbc, dbc[0:1, :])
                        nc.vector.tensor_mul(
                            out=oT[h // 2][(h % 2) * DH:(h % 2) * DH + DH,
                                           qs * QS:(qs + 1) * QS],
                            in0=o_ps[0:DH, :], in1=dbc)

                    def outproj_group(st, mc):
                        ps = pp.tile([P, QS], f32, tag="pj")
                        for ot in range(NOT):
                            nc.tensor.matmul(
                                ps,
                                lhsT=oT[ot][:, st * P:(st + 1) * P],
                                rhs=woT[ot][:, mc * QS:(mc + 1) * QS],
                                start=(ot == 0), stop=(ot == NOT - 1))
                        x_sk = lde.tile([P, QS], f32, tag="x_sk")
                        nc.sync.dma_start(
                            out=x_sk,
                            in_=x_d[st * P:(st + 1) * P,
                                    mc * QS:(mc + 1) * QS])
                        y_sb = tmpe.tile([P, QS], f32, tag="y_sb")
                        nc.vector.scalar_tensor_tensor(
                            out=y_sb, in0=x_sk, scalar=0.5, in1=ps,
                            op0=Alu.mult, op1=Alu.add)
                        nc.sync.dma_start(
                            out=out_d[st * P:(st + 1) * P,
                                      mc * QS:(mc + 1) * QS],
                            in_=y_sb)

                    pair_ctr = 0
                    for h0 in range(0, HL, 2):
                        h1 = h0 + 1
                        o_ps0 = op.tile([DH + 1, QS], f32, tag="o_ps")
                        o_ps1 = op.tile([DH + 1, QS], f32, tag="o_ps")
                        for kt0 in range(0, nkt, 2):
                            s0 = sp.tile([P, 2, QS], f32, tag="s_ps")
                            qk(h0, kt0, s0[:, 0, :])
                            qk(h0, kt0 + 1, s0[:, 1, :])
                            pt0 = expmask(kt0, s0)
                            s1 = sp.tile([P, 2, QS], f32, tag="s_ps")
                            qk(h1, kt0, s1[:, 0, :])
                            qk(h1, kt0 + 1, s1[:, 1, :])
                            pt1 = expmask(kt0, s1)
                            # prior q-super's output projection fills the
                            # PE gap while exp runs on the Scalar engine
                            if outproj_q and pair_ctr % 2 == 0:
                                outproj_q.pop(0)()
                            pair_ctr += 1
                            av(h0, kt0, pt0, o_ps0)
                            av(h1, kt0, pt1, o_ps1)
                        normalize(h0, o_ps0)
                        normalize(h1, o_ps1)

                    # queue this q-super's output projection; emitted inside
                    # the next chunk's attention to fill PE gaps
                    for st in range(qs * KT_PER_QS, (qs + 1) * KT_PER_QS):
                        for mc in range(2):
                            outproj_q.append(
                                lambda st=st, mc=mc: outproj_group(st, mc))

                # drain remaining output-projection work
                for w in outproj_q:
                    w()

    nc.compile()
    return nc


def _get_nc():
    if "nc" not in _CACHE:
        _CACHE["nc"] = _build_nc()
    return _CACHE["nc"]


def make_in_maps(x, key_val_lengths, Wq, bq, Wkv, bkv, Wo, bo, ln_g, ln_b):
    import ml_dtypes
    bf = ml_dtypes.bfloat16

    x = np.ascontiguousarray(np.asarray(x, dtype=np.float32))
    lens = np.asarray(key_val_lengths).astype(np.int64)
    Wq = np.asarray(Wq, dtype=np.float32)
    Wkv = np.asarray(Wkv, dtype=np.float32)
    Wo = np.asarray(Wo, dtype=np.float32)
    bq = np.asarray(bq, dtype=np.float32)
    bkv = np.asarray(bkv, dtype=np.float32)
    ln_g = np.asarray(ln_g, dtype=np.float32)
    ln_b = np.asarray(ln_b, dtype=np.float32)

    # fold LayerNorm gain into the projection weights and shift into the
    # biases (exact algebra): q = ((x-mu)rstd*g + b) @ Wq.T + bq
    #                           = xhat @ (Wq*g).T + (Wq@b + bq)
    g64 = ln_g.astype(np.float64)
    b64 = ln_b.astype(np.float64)
    Wq64 = Wq.astype(np.float64) * g64
    Wk64 = Wkv[:D].astype(np.float64) * g64
    Wv64 = Wkv[D:].astype(np.float64) * g64
    bq_f = (Wq.astype(np.float64) @ b64 + bq).astype(np.float32)
    bk_f = (Wkv[:D].astype(np.float64) @ b64 + bkv[:D]).astype(np.float32)
    bv_f = (Wkv[D:].astype(np.float64) @ b64 + bkv[D:]).astype(np.float32)

    in_maps = []
    for core in range(8):
        b, half = divmod(core, 2)
        sl = slice(half * OH, (half + 1) * OH)
        pad01 = (np.arange(S) < lens[b]).astype(np.float32)
        in_maps.append({
            "x": x[b],
            "wqt": np.ascontiguousarray(Wq64[sl].T.astype(bf)),
            "wkt": np.ascontiguousarray(Wk64[sl].T.astype(bf)),
            "wvt": np.ascontiguousarray(Wv64[sl].T.astype(bf)),
            "wot": np.ascontiguousarray(Wo[:, sl].T.astype(bf)),
            "bq": np.ascontiguousarray(bq_f[sl]),
            "bk": np.ascontiguousarray(bk_f[sl]),
            "bv": np.ascontiguousarray(bv_f[sl]),
            "pad01": np.ascontiguousarray(pad01.reshape(NST, P).T),
        })
    return in_maps


def kernel(**inputs):
    from concourse.bass_utils import run_bass_kernel_spmd

    trace = bool(os.environ.get("KERNEL_TRACE"))
    if trace:
        try:
            import antenv.axon_hooks  # noqa: F401  (profiling shim present?)
        except ImportError:
            trace = False
    nc = _get_nc()
    in_maps = make_in_maps(**inputs)
    res = run_bass_kernel_spmd(
        nc, in_maps, core_ids=list(range(8)), trace=trace)
    _CACHE["last_results"] = res
    bo = np.asarray(inputs["bo"], dtype=np.float32)
    y = np.empty((B, S, D), dtype=np.float32)
    for b in range(B):
        y[b] = res.results[2 * b]["out"] + res.results[2 * b + 1]["out"] + bo
    return y


# revision 37
# speedup vs baseline: 1.0406x; 1.0406x over previous
"""Trainium2 Bass kernel: causal multi-head self-attention block (pre-LN).

Full module computed on 8 NeuronCores:
    xn = LayerNorm(x); q = xn@Wq.T+bq; k,v = xn@Wkv.T+bkv
    out = softmax(mask(q k^T / sqrt(dh))) v @ Wo.T + bo + x

Sharding: core = batch_index * 2 + head_half.  Each core handles one batch
element and 8 of the 16 heads (column-parallel QKV, row-parallel Wo), emits a
partial [S, D] output including half the residual; host sums core pairs and
adds bo.  Weights are pre-transposed and cast to bf16 on the host so they DMA
straight into the matmul-ready layout.

Shapes are hardcoded for B=4, S=2048, D=1024, H=16, DH=64.
"""

import os
import sys

import numpy as np

sys.path.insert(0, "/opt/trn_rl_repo")

B, S, D, H = 4, 2048, 1024, 16
DH = D // H            # 64
HL = H // 2            # heads per core: 8
OH = HL * DH           # per-core head features: 512
EPS = 1e-5
NEG = -30000.0         # additive mask; exp(x + NEG) underflows to 0
P = 128                # SBUF partitions
NST = S // P           # 16 s-tiles
NFT = D // P           # 8 feature tiles
NOT = OH // P          # 4 o-tiles (per-core head features)
QS = 512               # query super-tile (matmul moving free dim)
NQS = S // QS          # 4
KT_PER_QS = QS // P    # 4 k-tiles per q-super

_CACHE = {}


def _build_nc():
    import concourse.bass as bass
    import concourse.bacc as bacc
    import concourse.tile as tile
    from concourse import mybir

    f32 = mybir.dt.float32
    bf16 = mybir.dt.bfloat16
    Alu = mybir.AluOpType
    Act = mybir.ActivationFunctionType

    nc = bacc.Bacc("TRN2", target_bir_lowering=False, debug=False, num_devices=8)

    # ---- DRAM I/O (per-core shard shapes; w* pre-transposed + bf16 on host) ----
    x_d = nc.dram_tensor("x", [S, D], f32, kind="ExternalInput").ap()
    wq_d = nc.dram_tensor("wqt", [D, OH], bf16, kind="ExternalInput").ap()
    wk_d = nc.dram_tensor("wkt", [D, OH], bf16, kind="ExternalInput").ap()
    wv_d = nc.dram_tensor("wvt", [D, OH], bf16, kind="ExternalInput").ap()
    wo_d = nc.dram_tensor("wot", [OH, D], bf16, kind="ExternalInput").ap()
    bq_d = nc.dram_tensor("bq", [OH], f32, kind="ExternalInput").ap()
    bk_d = nc.dram_tensor("bk", [OH], f32, kind="ExternalInput").ap()
    bv_d = nc.dram_tensor("bv", [OH], f32, kind="ExternalInput").ap()
    pad_d = nc.dram_tensor("pad01", [P, NST], f32, kind="ExternalInput").ap()
    out_d = nc.dram_tensor("out", [S, D], f32, kind="ExternalOutput").ap()
    debug = bool(os.environ.get("KERNEL_DEBUG"))
    if debug:
        dbg = {n: nc.dram_tensor(f"dbg_{n}", shp, bf16, kind="ExternalOutput").ap()
               for n, shp in (("xnT0", [P, S]), ("qT0", [P, S]), ("kT0", [P, S]),
                              ("vaug0", [P, HL * (DH + 1)]), ("oT0", [P, S]),
                              ("wqT0", [P, OH]))}

    def bcast(ap_1d, n):
        # [n] dram vector -> [P, n] partition-broadcast DMA source
        return bass.AP(tensor=ap_1d.tensor, offset=ap_1d.offset,
                       ap=[[0, P], [1, n]])

    with tile.TileContext(nc) as tc:
        with (
            tc.tile_pool(name="res", bufs=1) as res,       # resident tensors
            tc.tile_pool(name="small", bufs=4) as small,
        ):
            # ---------- constants ----------
            vb_sb = res.tile([P, OH], f32, tag="vb_sb")
            nc.sync.dma_start(out=vb_sb, in_=bcast(bv_d, OH))
            pad_sb = res.tile([P, NST], f32, tag="pad_sb")
            nc.sync.dma_start(out=pad_sb, in_=pad_d)
            zero_sb = res.tile([P, 1], f32, tag="zero_sb")
            nc.vector.memset(zero_sb, 0.0)
            ident_b = res.tile([P, P], bf16, tag="ident_b")
            nc.gpsimd.memset(ident_b, 0.0)
            nc.gpsimd.affine_select(
                out=ident_b, in_=ident_b, compare_op=Alu.not_equal, fill=1.0,
                base=0, pattern=[[-1, P]], channel_multiplier=1)
            bq_sb = res.tile([P, NOT], f32, tag="bq_sb")
            nc.sync.dma_start(out=bq_sb, in_=bq_d.rearrange("(t p) -> p t", p=P))
            bk_sb = res.tile([P, NOT], f32, tag="bk_sb")
            nc.sync.dma_start(out=bk_sb, in_=bk_d.rearrange("(t p) -> p t", p=P))
            eps_sb = res.tile([P, 1], f32, tag="eps_sb")
            nc.vector.memset(eps_sb, EPS)

            # ---------- resident big tensors ----------
            xnT = [res.tile([P, S], bf16, tag=f"xnT{j}", name=f"xnT{j}")
                   for j in range(NFT)]
            qT = [res.tile([P, S], bf16, tag=f"qT{t}", name=f"qT{t}")
                  for t in range(NOT)]
            kT = [res.tile([P, S], bf16, tag=f"kT{t}", name=f"kT{t}")
                  for t in range(NOT)]
            # V augmented with a ones column per head: [s, h*65 .. h*65+64]
            vaug = [res.tile([P, HL * (DH + 1)], bf16, tag=f"vaug{i}",
                             name=f"vaug{i}") for i in range(NST)]
            oT = [res.tile([P, S], bf16, tag=f"oT{t}", name=f"oT{t}")
                  for t in range(NOT)]
            wqT = [res.tile([P, OH], bf16, tag=f"wqT{j}", name=f"wqT{j}")
                   for j in range(NFT)]
            wkT = [res.tile([P, OH], bf16, tag=f"wkT{j}", name=f"wkT{j}")
                   for j in range(NFT)]
            wvT = [res.tile([P, OH], bf16, tag=f"wvT{j}", name=f"wvT{j}")
                   for j in range(NFT)]
            woT = [res.tile([P, D], bf16, tag=f"woT{t}", name=f"woT{t}")
                   for t in range(NOT)]

            # ---------- phases C/D/E interleaved ----------
            # s-chunk-major projections, then per-q-super attention + output
            # projection, so PE always has dense independent work in flight.
            with (
                tc.tile_pool(name="pj_psum", bufs=2, space="PSUM") as pp,
                tc.tile_pool(name="s_psum", bufs=2, space="PSUM") as sp,
                tc.tile_pool(name="o_psum", bufs=2, space="PSUM") as op,
                tc.tile_pool(name="pt", bufs=6) as ptp,
                tc.tile_pool(name="nrm", bufs=2) as nrm,
                tc.tile_pool(name="ld", bufs=3) as ld,
                tc.tile_pool(name="tmp", bufs=3) as tmp,
                tc.tile_pool(name="lde", bufs=3) as lde,
                tc.tile_pool(name="tmpe", bufs=3) as tmpe,
            ):
                for st in range(NST):
                    nc.gpsimd.memset(vaug[st], 1.0)
                def ln_compute(st, xns):
                    x_t = ld.tile([P, D], f32, tag="x_ln")
                    nc.sync.dma_start(out=x_t,
                                      in_=x_d[st * P:(st + 1) * P, :])
                    stats = small.tile([P, 2, 6], f32, tag="stats")
                    for sg in range(2):
                        nc.vector.bn_stats(
                            out=stats[:, sg, :],
                            in_=x_t[:, sg * 512:(sg + 1) * 512])
                    mv = small.tile([P, 2], f32, tag="mv")
                    nc.vector.bn_aggr(out=mv, in_=stats)
                    rstd = small.tile([P, 1], f32, tag="rstd")
                    nc.scalar.activation(out=rstd, in_=mv[:, 1:2],
                                         func=Act.Sqrt, bias=eps_sb,
                                         scale=1.0)
                    nc.vector.reciprocal(out=rstd, in_=rstd)
                    mb = small.tile([P, 1], f32, tag="mb")
                    nc.vector.tensor_scalar(
                        out=mb, in0=mv[:, 0:1], scalar1=rstd, scalar2=-1.0,
                        op0=Alu.mult, op1=Alu.mult)
                    xn = tmp.tile([P, D], bf16, tag="xn", bufs=9)
                    nc.scalar.activation(out=xn, in_=x_t, func=Act.Identity,
                                         bias=mb, scale=rstd)
                    xns[st] = xn

                def ln_transpose(st, xns):
                    xn = xns[st]
                    for j in range(NFT):
                        ps = pp.tile([P, P], bf16, tag="pj")
                        nc.tensor.transpose(
                            ps, xn[:, j * P:(j + 1) * P], ident_b)
                        nc.scalar.copy(
                            out=xnT[j][:, st * P:(st + 1) * P], in_=ps)

                xns = {}
                outproj_q = []
                for st in range(2 * KT_PER_QS):
                    ln_compute(st, xns)
                # weights DMA after the first LN x-loads so the prologue's
                # critical path isn't queued behind 5MB of weights
                for j in range(NFT):
                    nc.sync.dma_start(out=wqT[j], in_=wq_d[j * P:(j + 1) * P, :])
                    nc.sync.dma_start(out=wkT[j], in_=wk_d[j * P:(j + 1) * P, :])
                    nc.sync.dma_start(out=wvT[j], in_=wv_d[j * P:(j + 1) * P, :])
                for t in range(NOT):
                    nc.sync.dma_start(out=woT[t], in_=wo_d[t * P:(t + 1) * P, :])

                for c in range(NQS):
                    for st in range(c * KT_PER_QS, (c + 1) * KT_PER_QS):
                        ln_transpose(st, xns)
                    # --- projections for s-range [c*512, (c+1)*512) ---
                    for (wT, dst, bias) in ((wqT, qT, bq_sb), (wkT, kT, bk_sb)):
                        for t in range(NOT):
                            ps = pp.tile([P, QS], f32, tag="pj")
                            for j in range(NFT):
                                nc.tensor.matmul(
                                    ps,
                                    lhsT=wT[j][:, t * P:(t + 1) * P],
                                    rhs=xnT[j][:, c * QS:(c + 1) * QS],
                                    start=(j == 0), stop=(j == NFT - 1))
                            nc.vector.tensor_scalar_add(
                                out=dst[t][:, c * QS:(c + 1) * QS],
                                in0=ps, scalar1=bias[:, t:t + 1])
                    for st in range(c * KT_PER_QS, (c + 1) * KT_PER_QS):
                        ps = pp.tile([P, OH], f32, tag="pj")
                        for j in range(NFT):
                            nc.tensor.matmul(
                                ps,
                                lhsT=xnT[j][:, st * P:(st + 1) * P],
                                rhs=wvT[j],
                                start=(j == 0), stop=(j == NFT - 1))
                        for h in range(HL):
                            nc.vector.tensor_add(
                                out=vaug[st][:, h * (DH + 1):h * (DH + 1) + DH],
                                in0=ps[:, h * DH:(h + 1) * DH],
                                in1=vb_sb[:, h * DH:(h + 1) * DH])
                        nc.vector.tensor_scalar_mul(
                            out=vaug[st], in0=vaug[st],
                            scalar1=pad_sb[:, st:st + 1])

                    # LN for chunk c+2 races on DVE/GpSimd during attention
                    if c + 2 < NQS:
                        for st in range((c + 2) * KT_PER_QS,
                                        (c + 3) * KT_PER_QS):
                            ln_compute(st, xns)

                    # --- attention for qs=c, two heads' chains interleaved ---
                    qs = c
                    nkt = (qs + 1) * KT_PER_QS

                    def qk(h, kt, s_ps):
                        hq = qT[h // 2][(h % 2) * DH:(h % 2) * DH + DH, :]
                        hk = kT[h // 2][(h % 2) * DH:(h % 2) * DH + DH, :]
                        nc.tensor.matmul(
                            s_ps,
                            lhsT=hk[:, kt * P:(kt + 1) * P],
                            rhs=hq[:, qs * QS:(qs + 1) * QS],
                            start=True, stop=True, skip_group_check=True)

                    def expmask(kt0, s_ps):
                        pt = ptp.tile([P, 2, QS], bf16, tag="pt", bufs=4)
                        nc.scalar.activation(
                            out=pt, in_=s_ps, func=Act.Exp,
                            bias=zero_sb, scale=0.125)
                        for i in range(2):
                            kt = kt0 + i
                            if kt >= qs * KT_PER_QS:  # diagonal region
                                nc.gpsimd.affine_select(
                                    out=pt[:, i, :], in_=pt[:, i, :],
                                    compare_op=Alu.is_ge, fill=0.0,
                                    base=qs * QS - kt * P,
                                    pattern=[[1, QS]], channel_multiplier=-1)
                        return pt

                    def av(h, kt0, pt, o_ps):
                        for i in range(2):
                            kt = kt0 + i
                            nc.tensor.matmul(
                                o_ps,
                                lhsT=vaug[kt][:, h * (DH + 1):(h + 1) * (DH + 1)],
                                rhs=pt[:, i, :],
                                start=(kt == 0), stop=(kt == nkt - 1),
                                skip_group_check=True)

                    def normalize(h, o_ps):
                        den_sb = nrm.tile([1, QS], f32, tag="den_sb")
                        nc.vector.tensor_copy(den_sb, o_ps[DH:DH + 1, :])
                        dbc = nrm.tile([DH, QS], f32, tag="dbc")
                        nc.vector.reciprocal_approx_fast(
                            out=dbc[0:1, :], in_=den_sb)
                        nc.gpsimd.partition_broadcast(dbc, dbc[0:1, :])
                        nc.vector.tensor_mul(
                            out=oT[h // 2][(h % 2) * DH:(h % 2) * DH + DH,
                                           qs * QS:(qs + 1) * QS],
                            in0=o_ps[0:DH, :], in1=dbc)

                    def outproj_group(st, mc):
                        ps = pp.tile([P, QS], f32, tag="pj")
                        for ot in range(NOT):
                            nc.tensor.matmul(
                                ps,
                                lhsT=oT[ot][:, st * P:(st + 1) * P],
                                rhs=woT[ot][:, mc * QS:(mc + 1) * QS],
                                start=(ot == 0), stop=(ot == NOT - 1))
                        x_sk = lde.tile([P, QS], f32, tag="x_sk")
                        nc.sync.dma_start(
                            out=x_sk,
                            in_=x_d[st * P:(st + 1) * P,
                                    mc * QS:(mc + 1) * QS])
                        y_sb = tmpe.tile([P, QS], f32, tag="y_sb")
                        nc.vector.scalar_tensor_tensor(
                            out=y_sb, in0=x_sk, scalar=0.5, in1=ps,
                            op0=Alu.mult, op1=Alu.add)
                        nc.sync.dma_start(
                            out=out_d[st * P:(st + 1) * P,
                                      mc * QS:(mc + 1) * QS],
                            in_=y_sb)

                    for h0 in range(0, HL, 2):
                        h1 = h0 + 1
                        o_ps0 = op.tile([DH + 1, QS], f32, tag="o_ps")
                        o_ps1 = op.tile([DH + 1, QS], f32, tag="o_ps")
                        for kt0 in range(0, nkt, 2):
                            s0 = sp.tile([P, 2, QS], f32, tag="s_ps")
                            qk(h0, kt0, s0[:, 0, :])
                            qk(h0, kt0 + 1, s0[:, 1, :])
                            pt0 = expmask(kt0, s0)
                            s1 = sp.tile([P, 2, QS], f32, tag="s_ps")
                            qk(h1, kt0, s1[:, 0, :])
                            qk(h1, kt0 + 1, s1[:, 1, :])
                            pt1 = expmask(kt0, s1)
                            av(h0, kt0, pt0, o_ps0)
                            av(h1, kt0, pt1, o_ps1)
                        normalize(h0, o_ps0)
                        normalize(h1, o_ps1)
                        # previous q-super's output projection fills PE gaps
                        while outproj_q and len(outproj_q) > 2 * (HL - 2 - h0):
                            outproj_q.pop(0)()

                    # queue this q-super's output projection; emitted inside
                    # the next chunk's attention to fill PE gaps
                    for st in range(qs * KT_PER_QS, (qs + 1) * KT_PER_QS):
                        for mc in range(2):
                            outproj_q.append(
                                lambda st=st, mc=mc: outproj_group(st, mc))

                # drain remaining output-projection work
                for w in outproj_q:
                    w()

    nc.compile()
    return nc


def _get_nc():
    if "nc" not in _CACHE:
        _CACHE["nc"] = _build_nc()
    return _CACHE["nc"]


def make_in_maps(x, key_val_lengths, Wq, bq, Wkv, bkv, Wo, bo, ln_g, ln_b):
    import ml_dtypes
    bf = ml_dtypes.bfloat16

    x = np.ascontiguousarray(np.asarray(x, dtype=np.float32))
    lens = np.asarray(key_val_lengths).astype(np.int64)
    Wq = np.asarray(Wq, dtype=np.float32)
    Wkv = np.asarray(Wkv, dtype=np.float32)
    Wo = np.asarray(Wo, dtype=np.float32)
    bq = np.asarray(bq, dtype=np.float32)
    bkv = np.asarray(bkv, dtype=np.float32)
    ln_g = np.asarray(ln_g, dtype=np.float32)
    ln_b = np.asarray(ln_b, dtype=np.float32)

    # fold LayerNorm gain into the projection weights and shift into the
    # biases (exact algebra): q = ((x-mu)rstd*g + b) @ Wq.T + bq
    #                           = xhat @ (Wq*g).T + (Wq@b + bq)
    g64 = ln_g.astype(np.float64)
    b64 = ln_b.astype(np.float64)
    Wq64 = Wq.astype(np.float64) * g64
    Wk64 = Wkv[:D].astype(np.float64) * g64
    Wv64 = Wkv[D:].astype(np.float64) * g64
    bq_f = (Wq.astype(np.float64) @ b64 + bq).astype(np.float32)
    bk_f = (Wkv[:D].astype(np.float64) @ b64 + bkv[:D]).astype(np.float32)
    bv_f = (Wkv[D:].astype(np.float64) @ b64 + bkv[D:]).astype(np.float32)

    in_maps = []
    for core in range(8):
        b, half = divmod(core, 2)
        sl = slice(half * OH, (half + 1) * OH)
        pad01 = (np.arange(S) < lens[b]).astype(np.float32)
        in_maps.append({
            "x": x[b],
            "wqt": np.ascontiguousarray(Wq64[sl].T.astype(bf)),
            "wkt": np.ascontiguousarray(Wk64[sl].T.astype(bf)),
            "wvt": np.ascontiguousarray(Wv64[sl].T.astype(bf)),
            "wot": np.ascontiguousarray(Wo[:, sl].T.astype(bf)),
            "bq": np.ascontiguousarray(bq_f[sl]),
            "bk": np.ascontiguousarray(bk_f[sl]),
            "bv": np.ascontiguousarray(bv_f[sl]),
            "pad01": np.ascontiguousarray(pad01.reshape(NST, P).T),
        })
    return in_maps


def kernel(**inputs):
    from concourse.bass_utils import run_bass_kernel_spmd

    trace = bool(os.environ.get("KERNEL_TRACE"))
    if trace:
        try:
            import antenv.axon_hooks  # noqa: F401  (profiling shim present?)
        except ImportError:
            trace = False
    nc = _get_nc()
    in_maps = make_in_maps(**inputs)
    res = run_bass_kernel_spmd(
        nc, in_maps, core_ids=list(range(8)), trace=trace)
    _CACHE["last_results"] = res
    bo = np.asarray(inputs["bo"], dtype=np.float32)
    y = np.empty((B, S, D), dtype=np.float32)
    for b in range(B):
        y[b] = res.results[2 * b]["out"] + res.results[2 * b + 1]["out"] + bo
    return y
